# revision 39
# baseline (speedup 1.0000x reference)
"""Trainium2 Bass kernel for the ETD1 ODE block (nn_ODEblockW_28922309771809).

Math (mirrors the jax reference; 9 steps of IC <- L IC R + F regrouped as
3 strides of 3):
  X  = dt*A = diag(0.05*sigmoid(alpha)) @ (adj - I)    ||X||_2 ~ 0.05
  Xr = dt*B = 0.1*((w*clip(d,0,1)) @ w.T - I)          ||Xr||_2 ~ 0.18
  L^t and L^t@m2 are elementwise combos of powers of the SAME X, so the
  3-step forcing F3 = F + L F R + L^2 F R^2 is built collective-free from
  the replicated x0:
    U_p = X^p @ x0 (p=1,2);  P_t = 0.1*x0 + sum_p GC[t][p-1] U_p
    F3  = P0 + (P1 + P2@R)@R,   Z@R = Z + Z@ER   (delta form)
  EL3 = e^{3X} - I (deg 2), ER = e^{Xr} - I (deg 4), ER3 = e^{3Xr} - I
  (deg 6).  IC_{t+3} = S + S@ER3 + F3,  S = IC + EL3@IC -- 3 steps; only
  the two intermediate ICs are all-gathered (bf16); IC_0 = x replicated.

Distribution over 8 cores: node dim sharded 256 rows/core, feature dim
128/core for the R series. Four collectives total: one combined X|Xr
row-gather, one combined ER|ER3 row-gather, and the two IC gathers.
The gathered X stays resident in SBUF (partition-major contribution
layout -> contiguous loads); its storage is reused for the ER/ER3 rows
and the bf16 x0 chunks. All series/recurrence matmuls are bf16 with fp32
PSUM accumulation; the delta form keeps bf16 rounding scaled by
||e^. - I||. wmat runs in fp32r. Numpy bit-sim of this exact scheme:
1.06e-3 frob rel err vs the fp64 reference (gate 2e-2).
"""

import math
from contextlib import ExitStack

import numpy as np

import concourse.bass as bass
import concourse.mybir as mybir
import concourse.tile as tile
from concourse import bacc
from concourse.bass_utils import run_bass_kernel_spmd
from concourse.masks import make_identity

F32 = mybir.dt.float32
F32R = mybir.dt.float32r
BF16 = mybir.dt.bfloat16
AL = mybir.AluOpType
AF = mybir.ActivationFunctionType

N_CORES = 8
P = 128
N = 2048          # nodes
D = 1024          # features
RB = 256          # node rows per core
FBR = 128         # feature cols per core
NKC = N // P      # 16
DKC = D // P      # 8
RJ = RB // P      # 2
FH = 512          # free-dim half for row-form matmuls (1 PSUM bank)
XOFF = RJ * N     # column offset of the Xr row block in the combined gather

LGROUP = [list(range(N_CORES))]


def _gc(t, k):
    return 0.1 * ((t + 1) ** (k + 1) - t ** (k + 1)) / math.factorial(k + 1)


GC = [[_gc(t, k) for k in range(1, 3)] for t in range(3)]   # G_t deg-2 coeffs
EL3C = [3.0, 4.5]                                           # e^{3X}-I deg2
ER3C = [3.0, 4.5, 4.5, 3.375, 2.025, 1.0125]                # e^{3Xr}-I deg6


def build_nc():
    nc = bacc.Bacc("TRN2", target_bir_lowering=False, debug=False,
                   num_devices=N_CORES)

    # adj_rows is host-side pre-subtracted: adj[rows] - I[rows]
    adj_rows = nc.dram_tensor("adj_rows", [RB, N], F32, kind="ExternalInput")
    alpha_blk = nc.dram_tensor("alpha_blk", [RB], F32, kind="ExternalInput")
    x_full = nc.dram_tensor("x_full", [N, D], F32, kind="ExternalInput")
    x0_full = nc.dram_tensor("x0_full", [N, D], F32, kind="ExternalInput")
    x_locd = nc.dram_tensor("x_loc", [RB, D], F32, kind="ExternalInput")
    x0_locd = nc.dram_tensor("x0_loc", [RB, D], F32, kind="ExternalInput")
    wT_full = nc.dram_tensor("wT_full", [D, D], F32, kind="ExternalInput")
    wt_cols = nc.dram_tensor("wt_cols", [D, FBR], F32, kind="ExternalInput")
    eye_feat = nc.dram_tensor("eye_feat", [D, FBR], F32, kind="ExternalInput")
    d_full = nc.dram_tensor("d_full", [D], F32, kind="ExternalInput")
    z_loc = nc.dram_tensor("z_loc", [RB, D], F32, kind="ExternalOutput")

    with tile.TileContext(nc) as tc, ExitStack() as top:
        const = top.enter_context(tc.tile_pool(name="const", bufs=1))
        dram = top.enter_context(tc.tile_pool(name="dram", bufs=1, space="DRAM"))
        # PSUM banks (8): trf(2)+trb(2) persistent; psf: mmL(2)+mmR(2)
        # closed after the U passes; psr: f0..f3 (4) for W/V passes after.
        psum = top.enter_context(tc.tile_pool(name="psum", bufs=2, space="PSUM"))
        pf_st = ExitStack()
        psf = pf_st.enter_context(tc.tile_pool(name="psumf", bufs=2,
                                               space="PSUM"))
        slabp = top.enter_context(tc.tile_pool(name="slabp", bufs=1))
        scrp = top.enter_context(tc.tile_pool(name="scrp", bufs=1))
        lser = top.enter_context(tc.tile_pool(name="lser", bufs=1))
        rser = top.enter_context(tc.tile_pool(name="rser", bufs=1))
        recp = top.enter_context(tc.tile_pool(name="recp", bufs=1))

        ident = const.tile([P, P], F32)
        make_identity(nc, ident)
        ident_b = const.tile([P, P], BF16)
        nc.vector.tensor_copy(ident_b[:], ident[:])

        def pe_t(dst_slice, src_slice):
            """dst[128,128] = src[128,128].T via PE transpose (the PSUM->SBUF
            copy converts dtype if dst differs)."""
            if src_slice.dtype == F32R:
                src_slice = src_slice.bitcast(F32)
            bf = src_slice.dtype == BF16
            ps = psum.tile([P, P], BF16 if bf else F32,
                           tag="trb" if bf else "trf", bufs=2, name="ps_tr")
            nc.tensor.transpose(ps[:], src_slice, ident_b[:] if bf else ident[:])
            nc.vector.tensor_copy(dst_slice, ps[:])

        # =========================================================
        # Prep scales
        # =========================================================
        s_sb = const.tile([P, RJ], F32)
        nc.sync.dma_start(s_sb[:], alpha_blk.ap().rearrange("(j p) -> p j", p=P))
        nc.scalar.activation(s_sb[:], s_sb[:], AF.Sigmoid)
        nc.vector.tensor_scalar_mul(s_sb[:], s_sb[:], 0.05)

        d_sb = const.tile([P, DKC], F32)
        nc.sync.dma_start(d_sb[:], d_full.ap().rearrange("(q p) -> p q", p=P))
        nc.vector.tensor_scalar(d_sb[:], d_sb[:], 0.0, 1.0, AL.max, AL.min)

        # SBUF pool stack (LIFO): ph_ax (X cols, lives until F3) ->
        # rtmp (R series, until the er gather)
        pax_st = ExitStack()
        pax = pax_st.enter_context(tc.tile_pool(name="ph_ax", bufs=1))
        rt_st = ExitStack()
        rtmp = rt_st.enter_context(tc.tile_pool(name="rtmp", bufs=1))

        # one combined contribution [128, RJ*N + D]: X rows | Xr row block
        ccin_x = dram.tile([P, RJ * N + D], BF16, name="ccin_x")

        # =========================================================
        # R1: wmat col = (w diag(d)) @ wT[:, ccols] (fp32r, local), then
        # Xr col; its transposed row block goes into the combined ccin
        # =========================================================
        def t_out(ccin_slice, col_chunk):
            pr = scrp.tile([P, P], BF16, tag="prow", bufs=1, name="prow")
            pe_t(pr[:], col_chunk)
            nc.sync.dma_start(ccin_slice, pr[:])

        xrr = rtmp.tile([P, D], BF16, name="xrr")

        vr_ld = rtmp.tile([P, DKC, FBR], F32, tag="wslab", bufs=1,
                          name="vr_ld")
        nc.sync.dma_start(vr_ld[:],
                          wt_cols.ap().rearrange("(k p) n -> p k n", p=P))
        vr_sb = rtmp.tile([P, DKC, FBR], F32R, name="vr_sb")
        nc.vector.tensor_copy(vr_sb[:], vr_ld[:])
        xr_col = rtmp.tile([P, D], F32, tag="trow", bufs=1, name="xr_col")
        xr_b = rtmp.tile([P, DKC, FBR], BF16, name="xr_b")
        for m in range(DKC):
            wsl = rtmp.tile([P, DKC, FBR], F32, tag="wslab", bufs=1,
                            name="wslab")
            nc.sync.dma_start(
                wsl[:],
                wT_full[:, m * P:(m + 1) * P].rearrange("(k p) n -> p k n", p=P))
            wsr = rtmp.tile([P, DKC, FBR], F32R, tag="wsr", bufs=1, name="wsr")
            for k in range(DKC):
                nc.vector.tensor_scalar_mul(wsr[:, k, :], wsl[:, k, :],
                                            d_sb[:, k:k + 1])
            ps = psf.tile([P, FBR], F32, tag="mmR", bufs=2, name="ps_mmR")
            for k in range(DKC):
                nc.tensor.matmul(ps[:], wsr[:, k, :], vr_sb[:, k, :],
                                 start=(k == 0), stop=(k == DKC - 1))
            eyef = scrp.tile([P, FBR], F32, tag="eyef", bufs=1, name="eyef")
            nc.sync.dma_start(eyef[:], eye_feat[m * P:(m + 1) * P, :])
            xcs = xr_col[:, m * P:(m + 1) * P]
            nc.vector.tensor_sub(xcs, ps[:], eyef[:])
            nc.vector.tensor_scalar_mul(xcs, xcs, 0.1)
            nc.vector.tensor_copy(xr_b[:, m, :], xcs)
            pe_t(xrr[:, m * P:(m + 1) * P], xcs)
            nc.sync.dma_start(ccin_x[:, XOFF + m * P:XOFF + (m + 1) * P],
                              xrr[:, m * P:(m + 1) * P])

        # =========================================================
        # Phase A: X rows (bf16, streamed half-row chunks) into the
        # combined ccin + local transposes into xt_b; ONE AllGather of
        # [X | Xr]                                            [AG#1]
        # =========================================================
        HD = N // 4
        HT = NKC // 4
        xt_b = pax.tile([P, NKC, RB], BF16, name="xt_b")
        paxx_st = ExitStack()
        paxx = paxx_st.enter_context(tc.tile_pool(name="ph_axx", bufs=1))
        for j in range(RJ):
            for h in range(4):
                adj_sb = paxx.tile([P, HD], F32, tag="a_in", bufs=2,
                                   name="adj_sb")
                nc.sync.dma_start(adj_sb[:],
                                  adj_rows[j * P:(j + 1) * P,
                                           h * HD:(h + 1) * HD])
                xbh = paxx.tile([P, HD], BF16, tag="a_b", bufs=2, name="xbh")
                nc.vector.tensor_scalar_mul(xbh[:], adj_sb[:], s_sb[:, j:j + 1])
                nc.sync.dma_start(ccin_x[:, j * N + h * HD:
                                         j * N + (h + 1) * HD], xbh[:])
                for tt in range(HT):
                    pe_t(xt_b[:, h * HT + tt, j * P:(j + 1) * P],
                         xbh[:, tt * P:(tt + 1) * P])
        paxx_st.close()
        xfull_g = dram.tile([N_CORES * P, RJ * N + D], BF16,
                            addr_space="Shared", name="full_x")
        nc.gpsimd.collective_compute(
            "AllGather", AL.bypass, replica_groups=LGROUP,
            ins=[ccin_x.opt()], outs=[xfull_g.opt()])

        # resident gathered X: xsb[:, k, :] = X[kblk, :]  (64KB/partition;
        # rows 0..7 are later overwritten by ER|ER3 rows, rows 8..15 by
        # the bf16 x0 chunks -- same storage, three lives)
        xsb = rser.tile([P, NKC, N], BF16, name="xsb")
        for c in range(N_CORES):
            nc.sync.dma_start(
                xsb[:, c * RJ:(c + 1) * RJ, :],
                xfull_g[c * P:(c + 1) * P, 0:RJ * N].rearrange(
                    "p (j n) -> p j n", j=RJ))
        xr_sb = rtmp.tile([P, DKC, D], BF16, name="xr_sb")
        for c in range(N_CORES):
            nc.sync.dma_start(xr_sb[:, c, :],
                              xfull_g[c * P:(c + 1) * P, XOFF:XOFF + D])

        # ---- pass helpers ----
        def mm_passL(rhs_b, evict):
            for m in range(NKC):
                ps = psf.tile([P, RB], F32, tag="mmL", bufs=2, name="ps_mmL")
                for k in range(NKC):
                    nc.tensor.matmul(ps[:], xsb[:, k, m * P:(m + 1) * P],
                                     rhs_b[:, k, :],
                                     start=(k == 0), stop=(k == NKC - 1))
                evict(m, ps)

        def mm_rowR(lhs_cb, evict):
            """out rows [128, D] = sum_k lhs_cb[k].T @ Xr[kblk, :], in four
            FD-256 quarters on the mmL psum; evict(fq, ps)."""
            for fq in range(4):
                ps = psf.tile([P, RB], F32, tag="mmL", bufs=2, name="ps_mmR")
                for k in range(DKC):
                    nc.tensor.matmul(ps[:], lhs_cb[:, k, :],
                                     xr_sb[:, k, fq * RB:(fq + 1) * RB],
                                     start=(k == 0), stop=(k == DKC - 1))
                evict(fq, ps)

        def to_colb(row_b, dst_cb):
            for k in range(DKC):
                pe_t(dst_cb[:, k, :], row_b[:, k * P:(k + 1) * P])

        # =========================================================
        # X^2 pass (EL3 deg2 built at evict); then x0b fill, the R chain
        # and the combined er|er3 gather; U passes; F3.
        # =========================================================
        x2t_b = pax.tile([P, NKC, RB], BF16, name="x2t_b")
        elt3_b = lser.tile([P, NKC, RB], BF16, name="elt3_b")

        def ev_x2(m, ps):
            nc.vector.tensor_copy(x2t_b[:, m, :], ps[:])
            sc = scrp.tile([P, RB], F32, tag="combo", bufs=1, name="combo_scr")
            nc.vector.tensor_scalar_mul(sc[:], xt_b[:, m, :], EL3C[0])
            nc.vector.scalar_tensor_tensor(elt3_b[:, m, :], ps[:], EL3C[1],
                                           sc[:], AL.mult, AL.add)
        mm_passL(xt_b, ev_x2)

        # bf16 x0 chunks into xsb rows 8..15 (free once X^2's reads done)
        def stream_slot(k):
            return xsb[:, DKC + k // 2, (k % 2) * D:(k % 2 + 1) * D]

        for k in range(NKC):
            nc.gpsimd.dma_start(stream_slot(k),
                                x0_full[k * P:(k + 1) * P, :])

        # ---- R chain (row form, DMA-free): powers as row blocks with
        # col-chunk lhsT (symmetry: col chunk = transposed row chunk) ----
        ccin_er = dram.tile([P, 2 * D], BF16, name="ccin_er")
        xr2r = rtmp.tile([P, D], BF16, name="xr2r")
        xr3r = rtmp.tile([P, D], BF16, name="xr3r")
        xr4r = rtmp.tile([P, D], BF16, name="xr4r")
        def ev_row(dst):
            return lambda fq, ps: nc.vector.tensor_copy(
                dst[:, fq * RB:(fq + 1) * RB], ps[:])

        mm_rowR(xr_b, ev_row(xr2r))          # Xr^2 rows
        cb2 = rtmp.tile([P, DKC, P], BF16, tag="pwcb", bufs=2, name="cb2")
        to_colb(xr2r, cb2)
        mm_rowR(cb2, ev_row(xr3r))           # Xr^3 rows
        cb3 = rtmp.tile([P, DKC, P], BF16, tag="pwcb", bufs=2, name="cb3")
        to_colb(xr3r, cb3)
        mm_rowR(cb3, ev_row(xr4r))           # Xr^4 rows

        # T4 = Xr^2/6 + Xr^3/24 rows -> col chunks -> ER rows -> ccin
        t4r = rtmp.tile([P, D], F32, tag="trow", bufs=1, name="t4r")
        nc.vector.tensor_scalar_mul(t4r[:], xr2r[:], 1.0 / 6.0)
        nc.vector.scalar_tensor_tensor(t4r[:], xr3r[:], 1.0 / 24.0, t4r[:],
                                       AL.mult, AL.add)
        t4b = rtmp.tile([P, D], BF16, tag="trowb", bufs=1, name="t4b")
        nc.vector.tensor_copy(t4b[:], t4r[:])
        cbt = rtmp.tile([P, DKC, P], BF16, tag="pwcb", bufs=2, name="cbt4")
        to_colb(t4b, cbt)
        err_t = rtmp.tile([P, D], BF16, tag="errow", bufs=1, name="err_t")

        def ev_er(fq, ps):
            sl = slice(fq * RB, (fq + 1) * RB)
            sc = scrp.tile([P, RB], F32, tag="erc", bufs=1, name="er_scr")
            nc.vector.scalar_tensor_tensor(sc[:], xr2r[:, sl], 0.5, ps[:],
                                           AL.mult, AL.add)
            nc.vector.tensor_add(err_t[:, sl], sc[:], xrr[:, sl])
        mm_rowR(cbt, ev_er)
        nc.sync.dma_start(ccin_er[:, 0:D], err_t[:])

        # T6 = 3.375Xr^3 + 2.025Xr^4 + 1.0125Xr^5 -> ER3 rows -> ccin
        t6r = rtmp.tile([P, D], F32, tag="trow", bufs=1, name="t6r")
        nc.vector.tensor_scalar_mul(t6r[:], xr3r[:], ER3C[3])
        nc.vector.scalar_tensor_tensor(t6r[:], xr4r[:], ER3C[4], t6r[:],
                                       AL.mult, AL.add)
        t6b = rtmp.tile([P, D], BF16, tag="trowb", bufs=1, name="t6b")
        nc.vector.tensor_copy(t6b[:], t6r[:])
        cbt6 = rtmp.tile([P, DKC, P], BF16, tag="pwcb", bufs=2, name="cbt6")
        to_colb(t6b, cbt6)
        er3r_t = rtmp.tile([P, D], BF16, tag="errow", bufs=1, name="er3r_t")

        def ev_er3(fq, ps):
            sl = slice(fq * RB, (fq + 1) * RB)
            sc = scrp.tile([P, RB], F32, tag="erc", bufs=1, name="er3_scr")
            nc.vector.scalar_tensor_tensor(sc[:], xr2r[:, sl], ER3C[1], ps[:],
                                           AL.mult, AL.add)
            nc.vector.scalar_tensor_tensor(sc[:], xr3r[:, sl], ER3C[2], sc[:],
                                           AL.mult, AL.add)
            nc.vector.scalar_tensor_tensor(er3r_t[:, sl], xrr[:, sl], ER3C[0],
                                           sc[:], AL.mult, AL.add)
        mm_rowR(cbt6, ev_er3)
        nc.sync.dma_start(ccin_er[:, D:2 * D], er3r_t[:])

        er_g = dram.tile([N_CORES * P, 2 * D], BF16, addr_space="Shared",
                         name="full_er")
        nc.gpsimd.collective_compute(
            "AllGather", AL.bypass, replica_groups=LGROUP,
            ins=[ccin_er.opt()], outs=[er_g.opt()])
        rt_st.close()

        # =========================================================
        # U passes: U_p = X^p @ x0 (p=1,2; row form, FD=256 on mmL psum);
        # P_t = 0.1*x0 + GC[t].U is formed on the fly in the F3 build.
        # =========================================================
        pp_st = ExitStack()
        pp = pp_st.enter_context(tc.tile_pool(name="ph_p", bufs=1))
        x0_lc = pp.tile([P, RJ, D], F32, name="x0_lc")
        nc.sync.dma_start(x0_lc[:],
                          x0_locd.ap().rearrange("(j p) n -> p j n", p=P))
        u = [pp.tile([P, RJ, D], F32, name=f"u{p}") for p in range(2)]
        for p, lhs in enumerate((xt_b, x2t_b)):
            for j in range(RJ):
                for fq in range(4):
                    ps = psf.tile([P, RB], F32, tag="mmL", bufs=2,
                                  name="ps_mmL")
                    for k in range(NKC):
                        nc.tensor.matmul(
                            ps[:], lhs[:, k, j * P:(j + 1) * P],
                            stream_slot(k)[:, fq * RB:(fq + 1) * RB],
                            start=(k == 0), stop=(k == NKC - 1))
                    nc.vector.tensor_copy(
                        u[p][:, j, fq * RB:(fq + 1) * RB], ps[:])

        pf_st.close()
        psr = top.enter_context(tc.tile_pool(name="psumr", bufs=1,
                                             space="PSUM"))

        # er/er3 rows into xsb rows 0..7 (overwrite gathered X)
        for k in range(DKC):
            nc.sync.dma_start(xsb[:, k, 0:D], er_g[k * P:(k + 1) * P, 0:D])
            nc.sync.dma_start(xsb[:, k, D:2 * D],
                              er_g[k * P:(k + 1) * P, D:2 * D])

        # ---- W-type pass: out(j,f) = sum_k Z^T[k,j].T @ er[k, fslice];
        # er rows live in xsb[:, k, off:off+D] (off=0 -> ER, off=D -> ER3)
        def w_pass(zb_rows, er_off, evict):
            for j in range(RJ):
                zt = recp.tile([P, DKC, P], BF16, tag="zt", bufs=2,
                               name="zt_b")
                for k in range(DKC):
                    pe_t(zt[:, k, :], zb_rows[:, j, k * P:(k + 1) * P])
                pss = [psr.tile([P, FH], F32, tag=f"f{f}", bufs=1,
                                name=f"ps_f{f}") for f in range(2)]
                for k in range(DKC):
                    for f in range(2):
                        nc.tensor.matmul(
                            pss[f][:], zt[:, k, :],
                            xsb[:, k, er_off + f * FH:er_off + (f + 1) * FH],
                            start=(k == 0), stop=(k == DKC - 1))
                for f in range(2):
                    evict(j, f, pss[f])

        # ---- F3 = P0 + (P1 + P2@R)@R,  P_t from U_p on the fly ----
        def combo_p(dst, t, add_into=False):
            if add_into:
                nc.vector.scalar_tensor_tensor(dst[:], x0_lc[:], 0.1, dst[:],
                                               AL.mult, AL.add)
            else:
                nc.vector.tensor_scalar_mul(dst[:], x0_lc[:], 0.1)
            for p in range(2):
                nc.vector.scalar_tensor_tensor(dst[:], u[p][:], GC[t][p],
                                               dst[:], AL.mult, AL.add)

        q = pp.tile([P, RJ, D], F32, name="q_rows")
        tmp = recp.tile([P, RJ, D], F32, tag="s", bufs=1, name="p2_rows")
        combo_p(tmp, 2)
        qb = recp.tile([P, RJ, D], BF16, tag="qb", bufs=1, name="qb")
        nc.vector.tensor_copy(qb[:], tmp[:])
        w_pass(qb, 0,
               lambda j, f, ps: nc.vector.tensor_add(
                   q[:, j, f * FH:(f + 1) * FH], ps[:],
                   tmp[:, j, f * FH:(f + 1) * FH]))
        combo_p(q, 1, add_into=True)
        qb2 = recp.tile([P, RJ, D], BF16, tag="qb", bufs=1, name="qb2")
        nc.vector.tensor_copy(qb2[:], q[:])
        f3 = recp.tile([P, RJ, D], F32, name="f3_rows")

        def ev_f3(j, f, ps):
            sl = (slice(None), j, slice(f * FH, (f + 1) * FH))
            nc.vector.tensor_add(f3[sl], ps[:], q[sl])
        w_pass(qb2, 0, ev_f3)
        combo_p(f3, 0, add_into=True)
        pp_st.close()
        pax_st.close()

        # =========================================================
        # 3 recurrence steps: IC' = S + S@ER3 + F3,  S = IC + EL3@IC
        # =========================================================
        ic_g = [None, None]
        ic_state = [None]

        def step_ic(t):
            # V pass, k-outer; 4 psums (j,f). t=0 streams x with a bf16
            # convert hop; t>=1 streams the gathered IC on both DMA queues.
            s_rows = recp.tile([P, RJ, D], F32, tag="s", bufs=1, name="s_rows")
            if t == 0:
                nc.sync.dma_start(
                    s_rows[:], x_locd.ap().rearrange("(j p) n -> p j n", p=P))
            pss = [psr.tile([P, FH], F32, tag=f"f{i}", bufs=1,
                            name=f"ps_f{i}") for i in range(4)]
            for k in range(NKC):
                rkt = slabp.tile([P, D], BF16, tag="icc", bufs=3, name="cb")
                if t == 0:
                    nc.gpsimd.dma_start(rkt[:], x_full[k * P:(k + 1) * P, :])
                else:
                    eng = nc.sync if k % 2 == 0 else nc.scalar
                    eng.dma_start(rkt[:], ic_g[t - 1][k * P:(k + 1) * P, :])
                rk = rkt[:]
                for j in range(RJ):
                    for f in range(2):
                        nc.tensor.matmul(pss[2 * j + f][:],
                                         elt3_b[:, k, j * P:(j + 1) * P],
                                         rk[:, f * FH:(f + 1) * FH],
                                         start=(k == 0), stop=(k == NKC - 1))
            for j in range(RJ):
                for f in range(2):
                    sl = (slice(None), j, slice(f * FH, (f + 1) * FH))
                    if t == 0:
                        nc.vector.tensor_add(s_rows[sl], pss[2 * j + f][:],
                                             s_rows[sl])
                    else:
                        nc.vector.tensor_add(s_rows[sl], pss[2 * j + f][:],
                                             ic_state[0][sl])
            sb = recp.tile([P, RJ, D], BF16, tag="qb", bufs=1, name="sb")
            for j in range(RJ):
                nc.vector.tensor_copy(sb[:, j, :], s_rows[:, j, :])

            out = recp.tile([P, RJ, D], F32, tag="ic", bufs=2,
                            name="ic_rows" if t < 2 else "z_rows")

            def ev(j, f, ps):
                sl = (slice(None), j, slice(f * FH, (f + 1) * FH))
                nc.vector.tensor_add(out[sl], ps[:], s_rows[sl])
                nc.vector.tensor_add(out[sl], out[sl], f3[sl])
            w_pass(sb, D, ev)

            if t < 2:
                ic_state[0] = out
                ob = recp.tile([P, RJ, D], BF16, tag="qb", bufs=1, name="ob")
                ccin = dram.tile([RB, D], BF16, tag="ccin_ic",
                                 name=f"ccin_ic{t}")
                for j in range(RJ):
                    nc.vector.tensor_copy(ob[:, j, :], out[:, j, :])
                    nc.sync.dma_start(ccin[j * P:(j + 1) * P, :], ob[:, j, :])
                g = dram.tile([N, D], BF16, addr_space="Shared",
                              name=f"full_ic{t}")
                nc.gpsimd.collective_compute(
                    "AllGather", AL.bypass, replica_groups=LGROUP,
                    ins=[ccin.opt()], outs=[g.opt()])
                ic_g[t] = g
                # DMA-paced dummy matmuls keep the PE HAM warm through the
                # gather window (values unused; reads the settled X gather)
                for i in range(6):
                    wk = slabp.tile([P, D], BF16, tag="icc", bufs=3,
                                    name="warm_cb")
                    nc.gpsimd.dma_start(
                        wk[:], xfull_g[(i % 8) * P:(i % 8 + 1) * P, 0:D])
                    wp = psum.tile([P, P], F32, tag="trf", bufs=2,
                                   name="ps_warm")
                    nc.tensor.matmul(wp[:], ident_b[:], wk[:, 0:P],
                                     start=True, stop=True)
            else:
                for j in range(RJ):
                    nc.sync.dma_start(z_loc[j * P:(j + 1) * P, :], out[:, j, :])

        for t in range(3):
            step_ic(t)

    nc.compile()
    return nc


_NC_CACHE = []


def _get_nc():
    if not _NC_CACHE:
        _NC_CACHE.append(build_nc())
    return _NC_CACHE[0]


def make_in_maps(inputs):
    x = np.ascontiguousarray(np.asarray(inputs["x"], dtype=np.float32))
    x0 = np.ascontiguousarray(np.asarray(inputs["x0"], dtype=np.float32))
    adj = np.ascontiguousarray(np.asarray(inputs["adj"], dtype=np.float32))
    alpha = np.ascontiguousarray(
        np.asarray(inputs["alpha_train"], dtype=np.float32))
    w = np.ascontiguousarray(np.asarray(inputs["w"], dtype=np.float32))
    d = np.ascontiguousarray(np.asarray(inputs["d"], dtype=np.float32))

    eye_n = np.eye(N, dtype=np.float32)
    eye_d = np.eye(D, dtype=np.float32)
    wT = np.ascontiguousarray(w.T)

    in_maps = []
    for c in range(N_CORES):
        r0 = c * RB
        f0 = c * FBR
        in_maps.append({
            "adj_rows": np.ascontiguousarray(
                adj[r0:r0 + RB, :] - eye_n[r0:r0 + RB, :]),
            "alpha_blk": np.ascontiguousarray(alpha[r0:r0 + RB]),
            "x_full": x,
            "x0_full": x0,
            "x_loc": np.ascontiguousarray(x[r0:r0 + RB, :]),
            "x0_loc": np.ascontiguousarray(x0[r0:r0 + RB, :]),
            "wT_full": wT,
            "wt_cols": np.ascontiguousarray(wT[:, f0:f0 + FBR]),
            "eye_feat": np.ascontiguousarray(eye_d[:, f0:f0 + FBR]),
            "d_full": d,
        })
    return in_maps


def kernel(**inputs) -> np.ndarray:
    nc = _get_nc()
    in_maps = make_in_maps(inputs)
    res = run_bass_kernel_spmd(nc, in_maps, core_ids=list(range(N_CORES)))
    z = np.concatenate([res.results[c]["z_loc"] for c in range(N_CORES)],
                       axis=0)
    return np.ascontiguousarray(z.astype(np.float32))


if __name__ == "__main__":
    rng = np.random.default_rng(0)
    ins = {
        "x": rng.standard_normal((N, D)).astype(np.float32),
        "x0": rng.standard_normal((N, D)).astype(np.float32),
        "adj": (rng.random((N, N)) / N).astype(np.float32),
        "alpha_train": rng.standard_normal((N,)).astype(np.float32),
        "w": (np.eye(D) + 0.02 * rng.standard_normal((D, D))).astype(np.float32),
        "d": rng.random((D,)).astype(np.float32),
    }
    out = kernel(**ins)
    print("kernel output:", out.shape, out.dtype, float(np.linalg.norm(out)))
